# revision 1
# baseline (speedup 1.0000x reference)
"""Trainium2 Bass kernel for nn_Block_2302102471059 (ragged_sequence).

Pipeline per NeuronCore (8-way shard by GRU group ownership):
  - Each core owns 16 of the 128 sequence groups -> 8192 "slots"
    (slot s = l*16 + g_local, node id = seq_ids[g, l]).
  - Graph aggregation (segment mean over in-edges) is computed per owned
    slot directly from the replicated in_feats table: edges are bucketed
    by destination on the host, gathered 128-at-a-time with indirect DMA,
    and reduced with one-hot matmuls on the PE.
  - conv + ff1 run in a transposed layout (features on partitions,
    slots on the free dim), which is exactly what the GRU input matmul
    wants -- no layout changes anywhere in the main pipeline.
  - The GRU recurrence runs serially over L=512 steps with batch 16 in a
    transposed layout: h^T [128 part = 4 d-chunks x ..., 16 groups free].
    W_hh/W_ih live in SBUF as fp16 128x128 lhsT tiles (FWL-eligible).
  - ff2 + transpose back to row layout + contiguous store of a compact
    [8192, 512] fp32 per-core output; the host scatters rows into the
    full [65536, 512] output by seq_ids.

Everything on-device is fp16 storage with fp32 PSUM accumulation.
"""

import os
import sys

import numpy as np

sys.path.insert(0, "/opt/trn_rl_repo")

from contextlib import ExitStack

import concourse.bacc as bacc
import concourse.bass as bass
import concourse.tile as tile
from concourse import mybir
from concourse.bass import IndirectOffsetOnAxis
from concourse.bass_utils import run_bass_kernel_spmd
from concourse.masks import make_identity

N, D, E, G, L = 65536, 512, 1048576, 128, 512
NCORES = 8
GP = G // NCORES          # 16 groups per core
S = GP * L                # 8192 slots per core
NT = S // 128             # 64 dst-tiles of 128 slots
LCH = 32                  # l-steps per stream chunk
NCH = L // LCH            # 16 chunks
SCH = LCH * GP            # 512 slots per chunk
F16 = mybir.dt.float16
F32 = mybir.dt.float32
F32R = mybir.dt.float32r
I32 = mybir.dt.int32

LAST_RESULT = None
LAST_NC = None


def _build(nsub, debug=False):
    """Build the Bass program. nsub = padded 128-edge sub-blocks per dst-tile
    (uniform across cores/tiles; must be a multiple of 6)."""
    nc = bacc.Bacc("TRN2", target_bir_lowering=False, debug=False)
    calls_per_tile = nsub // 6

    # ---- DRAM I/O ----
    feats = nc.dram_tensor("in_feats", [N, D], F32, kind="ExternalInput")
    w_conv = nc.dram_tensor("W_conv", [D, D], F32, kind="ExternalInput")
    b_conv = nc.dram_tensor("b_conv", [D], F32, kind="ExternalInput")
    w_ff1 = nc.dram_tensor("W_ff1", [D, D], F32, kind="ExternalInput")
    b_ff1 = nc.dram_tensor("b_ff1", [D], F32, kind="ExternalInput")
    w_ih = nc.dram_tensor("W_ih", [3 * D, D], F32, kind="ExternalInput")
    w_hh = nc.dram_tensor("W_hh", [3 * D, D], F32, kind="ExternalInput")
    b_ih = nc.dram_tensor("b_ih", [3 * D], F32, kind="ExternalInput")
    b_hh = nc.dram_tensor("b_hh", [3 * D], F32, kind="ExternalInput")
    w_ff2 = nc.dram_tensor("W_ff2", [D, D], F32, kind="ExternalInput")
    b_ff2 = nc.dram_tensor("b_ff2", [D], F32, kind="ExternalInput")
    # Per-core index/meta arrays (host-prepared).
    # idx2d[p, t*nsub + q] = src node of padded edge slot (t, q, p)
    idx2d = nc.dram_tensor("idx2d", [128, NT * nsub], I32, kind="ExternalInput")
    ldst2d = nc.dram_tensor("ldst2d", [128, NT * nsub], I32, kind="ExternalInput")
    deg2d = nc.dram_tensor("deg2d", [128, NT], F32, kind="ExternalInput")
    out = nc.dram_tensor("out", [S, D], F32, kind="ExternalOutput")
    if debug:
        dbg_aggh = nc.dram_tensor("dbg_aggh", [128, 2048], F16, kind="ExternalOutput")
        dbg_mt = nc.dram_tensor("dbg_mt", [128, 2048], F16, kind="ExternalOutput")
        dbg_git = nc.dram_tensor("dbg_git", [128, LCH * 192], F16, kind="ExternalOutput")
        dbg_ring = nc.dram_tensor("dbg_ring", [128, 4096], F16, kind="ExternalOutput")

    with tile.TileContext(nc) as tc, ExitStack() as ctx:
        wpool = ctx.enter_context(tc.tile_pool(name="w", bufs=1))
        tmp = ctx.enter_context(tc.tile_pool(name="tmp", bufs=2))
        stage = ctx.enter_context(tc.tile_pool(name="stage", bufs=4))
        ohp = ctx.enter_context(tc.tile_pool(name="oh", bufs=2))
        aggp = ctx.enter_context(tc.tile_pool(name="agg", bufs=2))
        xtp = ctx.enter_context(tc.tile_pool(name="xt", bufs=2))
        ctp = ctx.enter_context(tc.tile_pool(name="ct", bufs=2))
        mtp = ctx.enter_context(tc.tile_pool(name="mt", bufs=2))
        gip = ctx.enter_context(tc.tile_pool(name="gi", bufs=2))
        grup = ctx.enter_context(tc.tile_pool(name="gru", bufs=2))
        outp = ctx.enter_context(tc.tile_pool(name="outw", bufs=2))
        ps_mm = ctx.enter_context(tc.tile_pool(name="psmm", bufs=3, space="PSUM"))
        ps_gi = ctx.enter_context(tc.tile_pool(name="psgi", bufs=1, space="PSUM"))
        ps_gru = ctx.enter_context(tc.tile_pool(name="psgru", bufs=1, space="PSUM"))
        ps_tr = ctx.enter_context(tc.tile_pool(name="pstr", bufs=2, space="PSUM"))

        # ---- constants / weights prep ----
        ident = wpool.tile([128, 128], F16, tag="ident")
        make_identity(nc, ident[:])
        iotaf = wpool.tile([128, 128], F32, tag="iotaf")
        iotai = tmp.tile([128, 128], I32, tag="ioi")
        nc.gpsimd.iota(iotai[:], pattern=[[1, 128]], base=0, channel_multiplier=0)
        nc.vector.tensor_copy(iotaf[:], iotai[:])
        zero64 = wpool.tile([128, 64], F16, tag="zero64")
        nc.vector.memset(zero64[:], 0.0)

        def load_cast(dram, rows, cols, tag):
            t = wpool.tile([128, cols * (rows // 128)], F16, tag=tag)
            for c in range(rows // 128):
                f = tmp.tile([128, cols], F32, tag="ldf")
                nc.gpsimd.dma_start(out=f[:], in_=dram[c * 128:(c + 1) * 128, :])
                nc.vector.tensor_copy(t[:, c * cols:(c + 1) * cols], f[:])
            return t

        # lhsT tiles: [k, m] with k = input feature. W_conv/W_ff1/W_ff2 are
        # already [in, out]; c-th row-block is the c-th k-chunk.
        wconv = load_cast(w_conv, D, D, "wconv")    # [:, c*512+m]
        wff1 = load_cast(w_ff1, D, D, "wff1")
        wff2 = load_cast(w_ff2, D, D, "wff2")

        def load_gate_T(dram, tag):
            # W [1536, 512] (gate-major rows) -> lhsT tiles [k=d, m=gate],
            # stored as [128, 4c * 1536] : col c*1536 + j*128 + m
            t = wpool.tile([128, 4 * 1536], F16, tag=tag)
            for j in range(12):
                f = tmp.tile([128, 512], F32, tag="ldf")
                nc.gpsimd.dma_start(out=f[:], in_=dram[j * 128:(j + 1) * 128, :])
                h = tmp.tile([128, 512], F16, tag="ldh")
                nc.vector.tensor_copy(h[:], f[:])
                for c in range(4):
                    pt = ps_tr.tile([128, 128], F16, space="PSUM", tag="ptr")
                    nc.tensor.transpose(pt[:], h[:, c * 128:(c + 1) * 128], ident[:])
                    nc.scalar.activation(
                        t[:, c * 1536 + j * 128: c * 1536 + (j + 1) * 128], pt[:],
                        mybir.ActivationFunctionType.Copy)
            return t

        wihT = load_gate_T(w_ih, "wihT")
        whhT = load_gate_T(w_hh, "whhT")

        def load_bias(dram, n, tag):
            # [n*128] -> [128, n]
            t = wpool.tile([128, n], F32, tag=tag)
            for m in range(n):
                nc.gpsimd.dma_start(out=t[:, m:m + 1],
                                  in_=dram[m * 128:(m + 1) * 128][:, None])
            return t

        bconv = load_bias(b_conv, 4, "bconv")
        bff1 = load_bias(b_ff1, 4, "bff1")
        bih = load_bias(b_ih, 12, "bih")
        bhh = load_bias(b_hh, 12, "bhh")
        bff2 = load_bias(b_ff2, 4, "bff2")

        # gate biases: the PSUM fuse adds gi (which already contains b_ih);
        # b_hh must be added too for r/z/n. Fold b_hh into the gi eviction
        # bias: total bias = b_ih + b_hh (both added to every step's gates).
        # NOTE: n-gate: reference computes inn + r*(hn) with hn including
        # b_hh_n. So b_hh_n must stay with gh_n (PSUM side), NOT folded.
        # r/z: sigmoid(gi_r + gh_r + b_ih_r + b_hh_r) -> fold b_hh_rz into
        # gi eviction; add b_hh_n to psum n-region via the gates path.
        bsum = wpool.tile([128, 12], F32, tag="bsum")
        nc.vector.tensor_add(bsum[:, 0:8], bih[:, 0:8], bhh[:, 0:8])
        nc.vector.tensor_copy(bsum[:, 8:12], bih[:, 8:12])
        # b_hh_n broadcast into h-layout [128, 64]: [p, c*16+b] = b_hh[1024+c*128+p]
        bhn = wpool.tile([128, 64], F32, tag="bhn")
        for c in range(4):
            bc = bhh[:, 8 + c:9 + c]
            b3 = bass.AP(bc.tensor, bc.offset, [bc.ap[0], [0, 16]])
            nc.vector.tensor_copy(bhn[:, c * 16:(c + 1) * 16], b3)

        # per-slot inverse degree
        degs = wpool.tile([128, NT], F32, tag="degs")
        nc.sync.dma_start(out=degs[:], in_=deg2d[:, :])
        degm = wpool.tile([128, NT], F32, tag="degm")
        nc.vector.tensor_scalar_max(degm[:], degs[:], 1.0)
        invdeg = wpool.tile([128, NT], F32, tag="invdeg")
        nc.vector.reciprocal(invdeg[:], degm[:])

        # edge meta
        idx_sb = wpool.tile([128, NT * nsub], I32, tag="idxsb")
        nc.sync.dma_start(out=idx_sb[:], in_=idx2d[:, :])
        ldst_i = tmp.tile([128, NT * nsub], I32, tag="ldsti")
        nc.sync.dma_start(out=ldst_i[:], in_=ldst2d[:, :])
        ldst_f = wpool.tile([128, NT * nsub], F32, tag="ldstf")
        nc.vector.tensor_copy(ldst_f[:], ldst_i[:])

        # GRU hidden ring buffer: 64 l-slots x [4 d-chunks x 16 groups]
        ring = wpool.tile([128, 64 * 64], F16, tag="ring")

        def gru_step(t_step, gi_t, gi_base):
            """One GRU step. gi_t: gi chunk tile; gi_base: col offset of this
            step's [128,192] block. Writes h_t into ring slot t_step%64."""
            if t_step == 0:
                h_prev = zero64[:]
            else:
                o = ((t_step - 1) % 64) * 64
                h_prev = ring[:, o:o + 64]
            ps = ps_gru.tile([128, 128], F32, space="PSUM", tag="psg")
            psn = ps_gru.tile([128, 64], F32, space="PSUM", tag="psn")
            # u_rz = gi_rz (+b_hh_rz folded already) ... identity matmul first
            nc.tensor.matmul(ps[:, 0:128], ident[:], gi_t[:, gi_base:gi_base + 128],
                             start=True, stop=False)
            for j in range(8):
                for c in range(4):
                    nc.tensor.matmul(
                        ps[:, j * 16:(j + 1) * 16],
                        whhT[:, c * 1536 + j * 128: c * 1536 + (j + 1) * 128],
                        h_prev[:, c * 16:(c + 1) * 16],
                        start=False, stop=(c == 3))
            for j in range(8, 12):
                for c in range(4):
                    nc.tensor.matmul(
                        psn[:, (j - 8) * 16:(j - 7) * 16],
                        whhT[:, c * 1536 + j * 128: c * 1536 + (j + 1) * 128],
                        h_prev[:, c * 16:(c + 1) * 16],
                        start=(c == 0), stop=(c == 3))
            # gates
            sig = grup.tile([128, 128], F16, tag="sig")
            nc.scalar.activation(sig[:], ps[:, 0:128],
                                 mybir.ActivationFunctionType.Sigmoid)
            # hn = gh_n + b_hh_n ; rhn = r * hn
            hn = grup.tile([128, 64], F16, tag="hn")
            nc.vector.tensor_add(hn[:], psn[:], bhn[:])
            rhn = grup.tile([128, 64], F16, tag="rhn")
            nc.vector.tensor_mul(rhn[:], sig[:, 0:64], hn[:])
            tg = grup.tile([128, 64], F16, tag="tg")
            nc.vector.tensor_add(tg[:], rhn[:], gi_t[:, gi_base + 128:gi_base + 192])
            n_t = grup.tile([128, 64], F16, tag="nt")
            nc.scalar.activation(n_t[:], tg[:], mybir.ActivationFunctionType.Tanh)
            zh = grup.tile([128, 64], F16, tag="zh")
            nc.vector.tensor_mul(zh[:], sig[:, 64:128], h_prev)
            omz = grup.tile([128, 64], F16, tag="omz")
            nc.scalar.activation(omz[:], sig[:, 64:128],
                                 mybir.ActivationFunctionType.Copy,
                                 bias=1.0, scale=-1.0)
            mm_ = grup.tile([128, 64], F16, tag="mm")
            nc.vector.tensor_mul(mm_[:], omz[:], n_t[:])
            hslot = ring[:, (t_step % 64) * 64:(t_step % 64) * 64 + 64]
            nc.vector.tensor_add(hslot, mm_[:], zh[:])

        def ff2_block(k):
            """slots [512k, 512k+512) = l in [32k, 32k+32); reads ring."""
            l0 = (LCH * k) % 64
            rr = ring[:].rearrange("p (l q) -> p l q", q=64)
            ot = outp.tile([128, 4 * 512], F16, tag="ot")
            for m in range(4):
                ps = ps_mm.tile([128, 512], F32, space="PSUM", tag="ps512")
                for c in range(4):
                    nc.tensor.matmul(
                        ps[:], wff2[:, c * 512 + m * 128: c * 512 + (m + 1) * 128],
                        rr[:, l0:l0 + LCH, c * 16:(c + 1) * 16],
                        start=(c == 0), stop=(c == 3))
                nc.scalar.activation(ot[:, m * 512:(m + 1) * 512], ps[:],
                                     mybir.ActivationFunctionType.Identity,
                                     bias=bff2[:, m:m + 1])
            for q in range(4):
                orow = outp.tile([128, 512], F32, tag="orow")
                for m in range(4):
                    pt = ps_tr.tile([128, 128], F16, space="PSUM", tag="ptr")
                    nc.tensor.transpose(pt[:], ot[:, m * 512 + q * 128: m * 512 + (q + 1) * 128],
                                        ident[:])
                    nc.vector.tensor_copy(orow[:, m * 128:(m + 1) * 128], pt[:])
                nc.sync.dma_start(out=out[k * 512 + q * 128: k * 512 + (q + 1) * 128, :],
                                  in_=orow[:])

        # ================= streaming main pipeline =================
        for k in range(NCH):
            # ---- aggregation for the chunk's 4 dst-tiles ----
            xt = xtp.tile([128, 4 * SCH], F16, tag="xt")  # [c*512 + s_local]
            for tt in range(4):
                tg = 4 * k + tt  # global dst-tile
                psa = ps_mm.tile([128, 512], F32, space="PSUM", tag="ps512")
                oh = ohp.tile([128, nsub * 128], F32R, tag="oh")
                for cc in range(calls_per_tile):
                    st = stage.tile([128, 6 * 512], F32R, tag="st")
                    for i6 in range(6):
                        qq = cc * 6 + i6
                        nc.gpsimd.indirect_dma_start(
                            out=st[:, i6 * 512:(i6 + 1) * 512], out_offset=None,
                            in_=feats[:, :],
                            in_offset=IndirectOffsetOnAxis(
                                ap=idx_sb[:, tg * nsub + qq: tg * nsub + qq + 1],
                                axis=0))
                    # one-hot for these 6 sub-blocks in one DVE op
                    src = ldst_f[:, tg * nsub + cc * 6: tg * nsub + cc * 6 + 6]
                    src3 = bass.AP(src.tensor, src.offset,
                                   [src.ap[0], src.ap[1], [0, 128]])
                    io3 = bass.AP(iotaf[:].tensor, iotaf[:].offset,
                                  [iotaf[:].ap[0], [0, 6], iotaf[:].ap[1]])
                    oh3 = oh[:, cc * 768:(cc + 1) * 768].rearrange(
                        "p (q m) -> p q m", m=128)
                    nc.vector.tensor_tensor(out=oh3, in0=src3, in1=io3,
                                            op=mybir.AluOpType.is_equal)
                    for i in range(6):
                        q = cc * 6 + i
                        nc.tensor.matmul(
                            psa[:], oh[:, q * 128:(q + 1) * 128],
                            st[:, i * 512:(i + 1) * 512],
                            start=(q == 0), stop=(q == nsub - 1))
                aggh = aggp.tile([128, 512], F16, tag="aggh")
                nc.vector.tensor_scalar(out=aggh[:], in0=psa[:],
                                        scalar1=invdeg[:, tg:tg + 1], scalar2=None,
                                        op0=mybir.AluOpType.mult)
                if debug and k == 0:
                    nc.gpsimd.dma_start(out=dbg_aggh[:, tt * 512:(tt + 1) * 512],
                                        in_=aggh[:])
                for c in range(4):
                    pt = ps_tr.tile([128, 128], F16, space="PSUM", tag="ptr")
                    nc.tensor.transpose(pt[:], aggh[:, c * 128:(c + 1) * 128], ident[:])
                    nc.scalar.activation(xt[:, c * 512 + tt * 128: c * 512 + (tt + 1) * 128],
                                         pt[:], mybir.ActivationFunctionType.Copy)
            # ---- conv ----
            ct = ctp.tile([128, 4 * SCH], F16, tag="ct")
            for m in range(4):
                ps = ps_mm.tile([128, 512], F32, space="PSUM", tag="ps512")
                for c in range(4):
                    nc.tensor.matmul(ps[:],
                                     wconv[:, c * 512 + m * 128: c * 512 + (m + 1) * 128],
                                     xt[:, c * 512:(c + 1) * 512],
                                     start=(c == 0), stop=(c == 3))
                nc.scalar.activation(ct[:, m * 512:(m + 1) * 512], ps[:],
                                     mybir.ActivationFunctionType.Identity,
                                     bias=bconv[:, m:m + 1])
            # ---- ff1 (relu) ----
            mt = mtp.tile([128, 4 * SCH], F16, tag="mt")
            for m in range(4):
                ps = ps_mm.tile([128, 512], F32, space="PSUM", tag="ps512")
                for c in range(4):
                    nc.tensor.matmul(ps[:],
                                     wff1[:, c * 512 + m * 128: c * 512 + (m + 1) * 128],
                                     ct[:, c * 512:(c + 1) * 512],
                                     start=(c == 0), stop=(c == 3))
                nc.scalar.activation(mt[:, m * 512:(m + 1) * 512], ps[:],
                                     mybir.ActivationFunctionType.Relu,
                                     bias=bff1[:, m:m + 1])
            if debug and k == 0:
                nc.gpsimd.dma_start(out=dbg_mt[:, :], in_=mt[:])
            # ---- gi for the chunk: [128, LCH*192], col l*192 + j*16 + b ----
            git = gip.tile([128, LCH * 192], F16, tag="git")
            gir = git[:].rearrange("p (l j b) -> p l j b", j=12, b=16)
            for rep in range(2):
                for j in range(12):
                    ps = ps_gi.tile([128, 256], F32, space="PSUM", tag="psgi")
                    for c in range(4):
                        nc.tensor.matmul(
                            ps[:], wihT[:, c * 1536 + j * 128: c * 1536 + (j + 1) * 128],
                            mt[:, c * 512 + rep * 256: c * 512 + rep * 256 + 256],
                            start=(c == 0), stop=(c == 3))
                    nc.scalar.activation(
                        gir[:, rep * 16:(rep + 1) * 16, j, :],
                        ps[:].rearrange("p (l b) -> p l b", b=16),
                        mybir.ActivationFunctionType.Identity,
                        bias=bsum[:, j:j + 1])
            if debug and k == 0:
                nc.gpsimd.dma_start(out=dbg_git[:, :], in_=git[:])
            # ---- 32 GRU steps ----
            for li in range(LCH):
                gru_step(k * LCH + li, git, li * 192)
            if debug and k == 1:
                nc.gpsimd.dma_start(out=dbg_ring[:, :], in_=ring[:])
            # ---- ff2 for the previous chunk's slots (ring safety: current
            # chunk k's ring writes are slots [32k..32k+32)%64; block k reads
            # the same — emit after steps so data is ready ----
            ff2_block(k)

    nc.compile()
    return nc


def _host_prep(inputs):
    """Bucket edges by destination slot per core; build per-core arrays."""
    seq_ids = np.asarray(inputs["seq_ids"]).astype(np.int64)
    edge_src = np.asarray(inputs["edge_src"]).astype(np.int64)
    edge_dst = np.asarray(inputs["edge_dst"]).astype(np.int64)

    counts = np.bincount(edge_dst, minlength=N)
    order = np.argsort(edge_dst, kind="stable")
    src_sorted = edge_src[order].astype(np.int32)
    rowptr = np.zeros(N + 1, dtype=np.int64)
    np.cumsum(counts, out=rowptr[1:])

    # slot -> node per core: slot s = l*GP + g_local
    # node = seq_ids[16c + g_local, l]
    slot_nodes = []
    for c in range(NCORES):
        sn = seq_ids[c * GP:(c + 1) * GP, :].T.reshape(-1)  # [S]
        slot_nodes.append(sn)

    # per dst-tile max edges -> nsub (uniform, multiple of 6)
    max_tile = 0
    tile_edges = []
    for c in range(NCORES):
        sn = slot_nodes[c]
        cnt = counts[sn]  # [S]
        te = cnt.reshape(NT, 128).sum(axis=1)
        tile_edges.append((cnt, te))
        max_tile = max(max_tile, int(te.max()))
    nsub = -(-max_tile // 768) * 6  # ceil to multiple of 6 sub-blocks
    nsub = max(nsub, 6)

    per_core = []
    for c in range(NCORES):
        sn = slot_nodes[c]
        cnt, te = tile_edges[c]
        idx2d = np.zeros((128, NT * nsub), dtype=np.int32)
        ldst2d = np.full((128, NT * nsub), 200, dtype=np.int32)
        for t in range(NT):
            nodes = sn[t * 128:(t + 1) * 128]
            k = int(te[t])
            srcs = np.empty(k, dtype=np.int32)
            ld = np.empty(k, dtype=np.int32)
            pos = 0
            for p in range(128):
                v = nodes[p]
                d = int(cnt[t * 128 + p])
                srcs[pos:pos + d] = src_sorted[rowptr[v]:rowptr[v] + d]
                ld[pos:pos + d] = p
                pos += d
            # pack into [p, q] with flat index q*128 + p
            npad = nsub * 128
            sp = np.zeros(npad, dtype=np.int32)
            lp = np.full(npad, 200, dtype=np.int32)
            sp[:k] = srcs
            lp[:k] = ld
            idx2d[:, t * nsub:(t + 1) * nsub] = sp.reshape(nsub, 128).T
            ldst2d[:, t * nsub:(t + 1) * nsub] = lp.reshape(nsub, 128).T
        deg2d = cnt.reshape(NT, 128).T.astype(np.float32)
        per_core.append({"idx2d": idx2d, "ldst2d": ldst2d, "deg2d": deg2d,
                         "slot_nodes": sn})
    return per_core, nsub


def kernel(**inputs):
    global LAST_RESULT, LAST_NC
    per_core, nsub = _host_prep(inputs)
    nc = _build(nsub)
    LAST_NC = nc

    shared = {}
    for name in ["in_feats", "W_conv", "b_conv", "W_ff1", "b_ff1", "W_ih",
                 "W_hh", "b_ih", "b_hh", "W_ff2", "b_ff2"]:
        shared[name] = np.ascontiguousarray(
            np.asarray(inputs[name]).astype(np.float32))

    in_maps = []
    for c in range(NCORES):
        m = dict(shared)
        m["idx2d"] = per_core[c]["idx2d"]
        m["ldst2d"] = per_core[c]["ldst2d"]
        m["deg2d"] = per_core[c]["deg2d"]
        in_maps.append(m)

    res = run_bass_kernel_spmd(nc, in_maps, list(range(NCORES)),
                               trace=bool(int(os.environ.get("KTRACE", "0"))))
    LAST_RESULT = res

    out_full = np.empty((N, D), dtype=np.float32)
    for c in range(NCORES):
        out_full[per_core[c]["slot_nodes"]] = res.results[c]["out"]
    return out_full



# revision 4
# speedup vs baseline: 1.2728x; 1.2728x over previous
"""Trainium2 Bass kernel for nn_Block_2302102471059 (ragged_sequence).

Pipeline per NeuronCore (8-way shard by GRU group ownership):
  - Each core owns 16 of the 128 sequence groups -> 8192 "slots"
    (slot s = l*16 + g_local, node id = seq_ids[g, l]).
  - Graph aggregation (segment mean over in-edges) per owned slot:
    in_feats is host-cast to f16 and split into 3 DRAM tables of <32768
    rows (dma_gather indices are int16); per dst-tile (128 slots) edges
    are bucketed by table on the host and fetched with one bulk
    dma_gather per (tile, table) (<=1024 rows per instruction = SWDGE
    ring capacity), then scatter-reduced into PSUM with one-hot matmuls.
  - The 1/deg mean scaling is folded into the layout transpose: instead
    of transposing with an identity, we matmul with diag(invdeg).
  - conv + ff1 in transposed layout (features on partitions), GRU over
    L=512 steps with batch 16, ff2 + transpose back, f16 stores; the
    host scatters rows into the full [65536, 512] f32 output.
"""

import os
import sys

import numpy as np

sys.path.insert(0, "/opt/trn_rl_repo")

from contextlib import ExitStack

import concourse.bacc as bacc
import concourse.bass as bass
import concourse.tile as tile
from concourse import mybir
from concourse.bass_utils import run_bass_kernel_spmd
from concourse.masks import make_identity

N, D, E, G, L = 65536, 512, 1048576, 128, 512
NCORES = 8
GP = G // NCORES          # 16 groups per core
S = GP * L                # 8192 slots per core
NT = S // 128             # 64 dst-tiles of 128 slots
LCH = 32                  # l-steps per stream chunk
NCH = L // LCH            # 16 chunks
F16 = mybir.dt.float16
F32 = mybir.dt.float32
I16 = mybir.dt.int16
I32 = mybir.dt.int32

# 3 gather tables (int16 index limit 32767)
TBASE = [0, 21846, 43691, 65536]
NTAB = 3

LAST_RESULT = None
LAST_NC = None


def _build(meta):
    """meta: dict with
      nblk[t][j]      blocks (128 rows) per (tile, table)
      tsizes[j]       rows per feats table
    """
    nblk = meta["nblk"]
    tsizes = meta["tsizes"]
    nblk_t = [sum(nblk[t]) for t in range(NT)]       # blocks per tile
    maxnblk = max(nblk_t)
    NBLKSUM = sum(nblk_t)                            # total ldst block-cols
    IDXW = 8 * NBLKSUM                               # idx cols (int16)
    # flat offsets
    blk_off = np.zeros((NT, NTAB), dtype=int)
    idx_off = np.zeros((NT, NTAB), dtype=int)
    acc = 0
    for t in range(NT):
        for j in range(NTAB):
            blk_off[t][j] = acc
            idx_off[t][j] = 8 * acc
            acc += nblk[t][j]

    nc = bacc.Bacc("TRN2", target_bir_lowering=False, debug=False)

    # ---- DRAM I/O ----
    feats = [nc.dram_tensor(f"feats{j}", [tsizes[j], D], F16, kind="ExternalInput")
             for j in range(NTAB)]
    w_conv = nc.dram_tensor("W_conv", [D, D], F32, kind="ExternalInput")
    b_conv = nc.dram_tensor("b_conv", [D], F32, kind="ExternalInput")
    w_ff1 = nc.dram_tensor("W_ff1", [D, D], F32, kind="ExternalInput")
    b_ff1 = nc.dram_tensor("b_ff1", [D], F32, kind="ExternalInput")
    w_ih = nc.dram_tensor("W_ih", [3 * D, D], F32, kind="ExternalInput")
    w_hh = nc.dram_tensor("W_hh", [3 * D, D], F32, kind="ExternalInput")
    b_ih = nc.dram_tensor("b_ih", [3 * D], F32, kind="ExternalInput")
    b_hh = nc.dram_tensor("b_hh", [3 * D], F32, kind="ExternalInput")
    w_ff2 = nc.dram_tensor("W_ff2", [D, D], F32, kind="ExternalInput")
    b_ff2 = nc.dram_tensor("b_ff2", [D], F32, kind="ExternalInput")
    idx2d = nc.dram_tensor("idx2d", [128, IDXW], I16, kind="ExternalInput")
    ldst2d = nc.dram_tensor("ldst2d", [128, NBLKSUM], F32, kind="ExternalInput")
    deg2d = nc.dram_tensor("deg2d", [128, NT], F32, kind="ExternalInput")
    out = nc.dram_tensor("out", [S, D], F16, kind="ExternalOutput")

    with tile.TileContext(nc) as tc, ExitStack() as ctx:
        wpool = ctx.enter_context(tc.tile_pool(name="w", bufs=1))
        tmp = ctx.enter_context(tc.tile_pool(name="tmp", bufs=2))
        stage = ctx.enter_context(tc.tile_pool(name="stage", bufs=2))
        ohp = ctx.enter_context(tc.tile_pool(name="oh", bufs=2))
        agghp = ctx.enter_context(tc.tile_pool(name="aggh", bufs=2))
        diagp = ctx.enter_context(tc.tile_pool(name="diag", bufs=2))
        xtp = ctx.enter_context(tc.tile_pool(name="xt", bufs=2))
        ctp = ctx.enter_context(tc.tile_pool(name="ct", bufs=2))
        mtp = ctx.enter_context(tc.tile_pool(name="mt", bufs=2))
        gip = ctx.enter_context(tc.tile_pool(name="gi", bufs=2))
        grup = ctx.enter_context(tc.tile_pool(name="gru", bufs=2))
        outp = ctx.enter_context(tc.tile_pool(name="outw", bufs=2))
        ps_a = ctx.enter_context(tc.tile_pool(name="psa", bufs=2, space="PSUM"))
        ps_mm = ctx.enter_context(tc.tile_pool(name="psmm", bufs=2, space="PSUM"))
        ps_gru = ctx.enter_context(tc.tile_pool(name="psgru", bufs=1, space="PSUM"))
        ps_tr = ctx.enter_context(tc.tile_pool(name="pstr", bufs=1, space="PSUM"))

        # ---- constants / weights prep ----
        ident = wpool.tile([128, 128], F16, tag="ident")
        make_identity(nc, ident[:])
        iotaf = wpool.tile([128, 128], F32, tag="iotaf")
        iotai = tmp.tile([128, 128], I32, tag="ioi")
        nc.gpsimd.iota(iotai[:], pattern=[[1, 128]], base=0, channel_multiplier=0)
        nc.vector.tensor_copy(iotaf[:], iotai[:])
        zero64 = wpool.tile([128, 64], F16, tag="zero64")
        nc.vector.memset(zero64[:], 0.0)

        def load_cast(dram, rows, cols, tag):
            t = wpool.tile([128, cols * (rows // 128)], F16, tag=tag)
            for c in range(rows // 128):
                f = tmp.tile([128, cols], F32, tag="ldf")
                nc.sync.dma_start(out=f[:], in_=dram[c * 128:(c + 1) * 128, :])
                nc.vector.tensor_copy(t[:, c * cols:(c + 1) * cols], f[:])
            return t

        # lhsT tiles: [k, m] with k = input feature.
        wconv = load_cast(w_conv, D, D, "wconv")
        wff1 = load_cast(w_ff1, D, D, "wff1")
        wff2 = load_cast(w_ff2, D, D, "wff2")

        def load_gate_T(dram, tag):
            # W [1536, 512] (gate-major rows) -> lhsT tiles [k=d, m=gate],
            # stored as [128, 4c * 1536] : col c*1536 + j*128 + m
            t = wpool.tile([128, 4 * 1536], F16, tag=tag)
            for j in range(12):
                f = tmp.tile([128, 512], F32, tag="ldf")
                nc.sync.dma_start(out=f[:], in_=dram[j * 128:(j + 1) * 128, :])
                h = tmp.tile([128, 512], F16, tag="ldh")
                nc.vector.tensor_copy(h[:], f[:])
                pt = ps_tr.tile([128, 512], F16, space="PSUM", tag="ptr")
                for c in range(4):
                    nc.tensor.transpose(pt[:, c * 128:(c + 1) * 128],
                                        h[:, c * 128:(c + 1) * 128], ident[:])
                tv = t[:].rearrange("p (c m) -> p c m", m=1536)
                nc.scalar.activation(
                    tv[:, :, j * 128:(j + 1) * 128],
                    pt[:].rearrange("p (c m) -> p c m", m=128),
                    mybir.ActivationFunctionType.Copy)
            return t

        wihT = load_gate_T(w_ih, "wihT")
        whhT = load_gate_T(w_hh, "whhT")

        def load_bias(dram, n, tag):
            # [n*128] -> [128, n]
            t = wpool.tile([128, n], F32, tag=tag)
            for m in range(n):
                nc.sync.dma_start(out=t[:, m:m + 1],
                                  in_=dram[m * 128:(m + 1) * 128][:, None])
            return t

        bconv = load_bias(b_conv, 4, "bconv")
        bff1 = load_bias(b_ff1, 4, "bff1")
        bih = load_bias(b_ih, 12, "bih")
        bhh = load_bias(b_hh, 12, "bhh")
        bff2 = load_bias(b_ff2, 4, "bff2")

        # gi eviction bias: b_ih + b_hh for r/z (both outside the r-mult);
        # b_ih only for n (b_hh_n stays inside r*(...): folded into psn via
        # an identity matmul of bhn below).
        bsum = wpool.tile([128, 12], F32, tag="bsum")
        nc.vector.tensor_add(bsum[:, 0:8], bih[:, 0:8], bhh[:, 0:8])
        nc.vector.tensor_copy(bsum[:, 8:12], bih[:, 8:12])
        # b_hh_n broadcast in psn layout [128, 64]: [p, c*16+b] = b_hh[1024+c*128+p]
        bhn = wpool.tile([128, 64], F16, tag="bhn")
        for c in range(4):
            bc = bhh[:, 8 + c:9 + c]
            b3 = bass.AP(bc.tensor, bc.offset, [bc.ap[0], [0, 16]])
            nc.vector.tensor_copy(bhn[:, c * 16:(c + 1) * 16], b3)

        # per-slot inverse degree
        degs = wpool.tile([128, NT], F32, tag="degs")
        nc.sync.dma_start(out=degs[:], in_=deg2d[:, :])
        degm = wpool.tile([128, NT], F32, tag="degm")
        nc.vector.tensor_scalar_max(degm[:], degs[:], 1.0)
        invdeg = wpool.tile([128, NT], F32, tag="invdeg")
        nc.vector.reciprocal(invdeg[:], degm[:])

        # edge meta
        idx_sb = wpool.tile([128, IDXW], I16, tag="idxsb")
        nc.sync.dma_start(out=idx_sb[:], in_=idx2d[:, :])
        ldst_f = wpool.tile([128, NBLKSUM], F32, tag="ldstf")
        nc.sync.dma_start(out=ldst_f[:], in_=ldst2d[:, :])

        # GRU hidden ring buffer: 64 l-slots x [4 d-chunks x 16 groups]
        ring = wpool.tile([128, 64 * 64], F16, tag="ring")

        def gather_tile(t):
            """Issue the 3 dma_gathers for tile t into a staging tile;
            returns (st, nb) with blocks of the 3 tables back to back."""
            nb = nblk_t[t]
            st = stage.tile([128, maxnblk * D], F16, tag="st")
            b0 = 0
            for j in range(NTAB):
                nbj = nblk[t][j]
                if nbj == 0:
                    continue
                nidx = 128 * nbj
                nc.gpsimd.dma_gather(
                    out_ap=st[:, b0 * D:(b0 + nbj) * D].rearrange(
                        "p (b c) -> p b c", c=D),
                    in_ap=feats[j][:, :],
                    idxs_ap=idx_sb[:, idx_off[t][j]: idx_off[t][j] + 8 * nbj],
                    num_idxs=nidx,
                    num_idxs_reg=nidx,
                    elem_size=D,
                )
                b0 += nbj
            return st, nb

        def agg_tile(t, st, xt, tt):
            """One-hot scatter-reduce tile t from staging st, then
            diag(invdeg)-scaled transpose into xt[:, c*512 + tt*128]."""
            nb = nblk_t[t]
            psa = ps_a.tile([128, 512], F32, space="PSUM", tag="psa")
            for g0 in range(0, nb, 6):
                gn = min(6, nb - g0)
                oh = ohp.tile([128, 6 * 128], F16, tag="oh")
                src = ldst_f[:, blk_off[t][0] + g0: blk_off[t][0] + g0 + gn]
                src3 = bass.AP(src.tensor, src.offset,
                               [src.ap[0], src.ap[1], [0, 128]])
                io3 = bass.AP(iotaf[:].tensor, iotaf[:].offset,
                              [iotaf[:].ap[0], [0, gn], iotaf[:].ap[1]])
                oh3 = oh[:, 0:gn * 128].rearrange("p (q m) -> p q m", m=128)
                nc.vector.tensor_tensor(out=oh3, in0=src3, in1=io3,
                                        op=mybir.AluOpType.is_equal)
                for i in range(gn):
                    b = g0 + i
                    nc.tensor.matmul(
                        psa[:], oh[:, i * 128:(i + 1) * 128],
                        st[:, b * D:(b + 1) * D],
                        start=(b == 0), stop=(b == nb - 1))
            aggh = agghp.tile([128, 512], F16, tag="aggh")
            nc.scalar.activation(aggh[:], psa[:],
                                 mybir.ActivationFunctionType.Copy)
            diag = diagp.tile([128, 128], F16, tag="diag")
            nc.vector.tensor_scalar(out=diag[:], in0=ident[:],
                                    scalar1=invdeg[:, t:t + 1], scalar2=None,
                                    op0=mybir.AluOpType.mult)
            pt = ps_tr.tile([128, 512], F32, space="PSUM", tag="ptr32")
            for c in range(4):
                nc.tensor.matmul(pt[:, c * 128:(c + 1) * 128],
                                 aggh[:, c * 128:(c + 1) * 128], diag[:],
                                 start=True, stop=True)
            xv = xt[:].rearrange("p (c m) -> p c m", m=512)
            nc.scalar.activation(xv[:, :, tt * 128:(tt + 1) * 128],
                                 pt[:].rearrange("p (c m) -> p c m", m=128),
                                 mybir.ActivationFunctionType.Copy)

        def gru_step(t_step, gi_t, gi_base):
            """One GRU step. gi_t: gi chunk tile; gi_base: col offset of this
            step's [128,192] block. Writes h_t into ring slot t_step%64."""
            if t_step == 0:
                h_prev = zero64[:]
            else:
                o = ((t_step - 1) % 64) * 64
                h_prev = ring[:, o:o + 64]
            psb = ps_gru.tile([128, 192], F32, space="PSUM", tag="psg")
            ps = psb[:, 0:128]
            psn = psb[:, 128:192]
            # rz: psum = gi_rz (+b_ih+b_hh folded at eviction) + W_hh_rz h
            nc.tensor.matmul(ps[:, 0:128], ident[:], gi_t[:, gi_base:gi_base + 128],
                             start=True, stop=False)
            for j in range(8):
                for c in range(4):
                    nc.tensor.matmul(
                        ps[:, j * 16:(j + 1) * 16],
                        whhT[:, c * 1536 + j * 128: c * 1536 + (j + 1) * 128],
                        h_prev[:, c * 16:(c + 1) * 16],
                        start=False, stop=(c == 3))
            # n: psn = b_hh_n + W_hh_n h
            nc.tensor.matmul(psn, ident[:], bhn[:],
                             start=True, stop=False)
            for j in range(8, 12):
                for c in range(4):
                    nc.tensor.matmul(
                        psb[:, 128 + (j - 8) * 16:128 + (j - 7) * 16],
                        whhT[:, c * 1536 + j * 128: c * 1536 + (j + 1) * 128],
                        h_prev[:, c * 16:(c + 1) * 16],
                        start=False, stop=(c == 3))
            sig = grup.tile([128, 128], F16, tag="sig")
            nc.scalar.activation(sig[:], ps[:, 0:128],
                                 mybir.ActivationFunctionType.Sigmoid)
            rhn = grup.tile([128, 64], F16, tag="rhn")
            nc.vector.tensor_mul(rhn[:], sig[:, 0:64], psn)
            tg = grup.tile([128, 64], F16, tag="tg")
            nc.vector.tensor_add(tg[:], rhn[:], gi_t[:, gi_base + 128:gi_base + 192])
            n_t = grup.tile([128, 64], F16, tag="nt")
            nc.scalar.activation(n_t[:], tg[:], mybir.ActivationFunctionType.Tanh)
            zh = grup.tile([128, 64], F16, tag="zh")
            nc.vector.tensor_mul(zh[:], sig[:, 64:128], h_prev)
            omz = grup.tile([128, 64], F16, tag="omz")
            nc.gpsimd.tensor_scalar(out=omz[:], in0=sig[:, 64:128],
                                    scalar1=-1.0, scalar2=1.0,
                                    op0=mybir.AluOpType.mult,
                                    op1=mybir.AluOpType.add)
            mm_ = grup.tile([128, 64], F16, tag="mm")
            nc.vector.tensor_mul(mm_[:], omz[:], n_t[:])
            hslot = ring[:, (t_step % 64) * 64:(t_step % 64) * 64 + 64]
            nc.vector.tensor_add(hslot, mm_[:], zh[:])

        def ff2_block(k):
            """slots [512k, 512k+512) = l in [32k, 32k+32); reads ring."""
            l0 = (LCH * k) % 64
            rr = ring[:].rearrange("p (l q) -> p l q", q=64)
            ot = outp.tile([128, 4 * 512], F16, tag="ot")
            for m in range(4):
                ps = ps_mm.tile([128, 512], F32, space="PSUM", tag="ps512")
                for c in range(4):
                    nc.tensor.matmul(
                        ps[:], wff2[:, c * 512 + m * 128: c * 512 + (m + 1) * 128],
                        rr[:, l0:l0 + LCH, c * 16:(c + 1) * 16],
                        start=(c == 0), stop=(c == 3))
                nc.scalar.activation(ot[:, m * 512:(m + 1) * 512], ps[:],
                                     mybir.ActivationFunctionType.Identity,
                                     bias=bff2[:, m:m + 1])
            for q in range(4):
                orow = outp.tile([128, 512], F16, tag="orow")
                pt = ps_tr.tile([128, 512], F16, space="PSUM", tag="ptr")
                for m in range(4):
                    nc.tensor.transpose(pt[:, m * 128:(m + 1) * 128],
                                        ot[:, m * 512 + q * 128: m * 512 + (q + 1) * 128],
                                        ident[:])
                nc.vector.tensor_copy(orow[:], pt[:])
                nc.sync.dma_start(out=out[k * 512 + q * 128: k * 512 + (q + 1) * 128, :],
                                  in_=orow[:])

        # ================= streaming main pipeline =================
        for k in range(NCH):
            xt = xtp.tile([128, 4 * 512], F16, tag="xt")  # [c*512 + s_local]
            for tt in range(4):
                t = 4 * k + tt
                st, nb = gather_tile(t)
                agg_tile(t, st, xt, tt)
            # ---- conv ----
            ct = ctp.tile([128, 4 * 512], F16, tag="ct")
            for m in range(4):
                ps = ps_mm.tile([128, 512], F32, space="PSUM", tag="ps512")
                for c in range(4):
                    nc.tensor.matmul(ps[:],
                                     wconv[:, c * 512 + m * 128: c * 512 + (m + 1) * 128],
                                     xt[:, c * 512:(c + 1) * 512],
                                     start=(c == 0), stop=(c == 3))
                nc.scalar.activation(ct[:, m * 512:(m + 1) * 512], ps[:],
                                     mybir.ActivationFunctionType.Identity,
                                     bias=bconv[:, m:m + 1])
            # ---- ff1 (relu) ----
            mt = mtp.tile([128, 4 * 512], F16, tag="mt")
            for m in range(4):
                ps = ps_mm.tile([128, 512], F32, space="PSUM", tag="ps512")
                for c in range(4):
                    nc.tensor.matmul(ps[:],
                                     wff1[:, c * 512 + m * 128: c * 512 + (m + 1) * 128],
                                     ct[:, c * 512:(c + 1) * 512],
                                     start=(c == 0), stop=(c == 3))
                nc.scalar.activation(mt[:, m * 512:(m + 1) * 512], ps[:],
                                     mybir.ActivationFunctionType.Relu,
                                     bias=bff1[:, m:m + 1])
            # ---- gi: [128, LCH*192], col l*192 + j*16 + b ----
            git = gip.tile([128, LCH * 192], F16, tag="git")
            gir = git[:].rearrange("p (l j b) -> p l j b", j=12, b=16)
            for j in range(12):
                ps = ps_mm.tile([128, 512], F32, space="PSUM", tag="ps512")
                for c in range(4):
                    nc.tensor.matmul(
                        ps[:], wihT[:, c * 1536 + j * 128: c * 1536 + (j + 1) * 128],
                        mt[:, c * 512:(c + 1) * 512],
                        start=(c == 0), stop=(c == 3))
                nc.scalar.activation(
                    gir[:, :, j, :],
                    ps[:].rearrange("p (l b) -> p l b", b=16),
                    mybir.ActivationFunctionType.Identity,
                    bias=bsum[:, j:j + 1])
            # ---- 32 GRU steps ----
            for li in range(LCH):
                gru_step(k * LCH + li, git, li * 192)
            # ---- ff2 for this chunk's slots ----
            ff2_block(k)

    nc.compile()
    return nc


def _host_prep(inputs):
    """Bucket edges by (dst tile, src table) per core; build per-core arrays."""
    seq_ids = np.asarray(inputs["seq_ids"]).astype(np.int64)
    edge_src = np.asarray(inputs["edge_src"]).astype(np.int64)
    edge_dst = np.asarray(inputs["edge_dst"]).astype(np.int64)

    counts = np.bincount(edge_dst, minlength=N)
    order = np.argsort(edge_dst, kind="stable")
    src_sorted = edge_src[order].astype(np.int32)
    rowptr = np.zeros(N + 1, dtype=np.int64)
    np.cumsum(counts, out=rowptr[1:])

    tb = np.asarray(TBASE)

    per_core_raw = []
    cnt_blocks = np.zeros((NCORES, NT, NTAB), dtype=np.int64)
    for c in range(NCORES):
        sn = seq_ids[c * GP:(c + 1) * GP, :].T.reshape(-1)  # [S] slot->node
        deg = counts[sn]
        starts = rowptr[sn]
        tot = int(deg.sum())
        # segmented arange gather of all edges in slot order
        csum = np.cumsum(deg) - deg
        seg = np.arange(tot, dtype=np.int64) - np.repeat(csum, deg)
        esrc = src_sorted[np.repeat(starts, deg) + seg]
        slot_ids = np.repeat(np.arange(S, dtype=np.int64), deg)
        eldst = (slot_ids % 128).astype(np.int32)
        etile = slot_ids // 128
        etab = np.searchsorted(tb[1:NTAB], esrc, side="right")
        key = etile * NTAB + etab
        o2 = np.argsort(key, kind="stable")
        esrc_l = (esrc[o2] - tb[etab[o2]]).astype(np.int16)
        eldst_s = eldst[o2]
        key_s = key[o2]
        kcnt = np.bincount(key_s, minlength=NT * NTAB).reshape(NT, NTAB)
        cnt_blocks[c] = (kcnt + 127) // 128
        per_core_raw.append((sn, deg, esrc_l, eldst_s, kcnt))

    nblk = cnt_blocks.max(axis=0)  # [NT, NTAB]
    nblk_t = nblk.sum(axis=1)
    NBLKSUM = int(nblk_t.sum())
    IDXW = 8 * NBLKSUM

    per_core = []
    for c in range(NCORES):
        sn, deg, esrc_l, eldst_s, kcnt = per_core_raw[c]
        kptr = np.zeros(NT * NTAB + 1, dtype=np.int64)
        np.cumsum(kcnt.reshape(-1), out=kptr[1:])
        idx2d = np.zeros((128, IDXW), dtype=np.int16)
        ldst2d = np.full((128, NBLKSUM), 200.0, dtype=np.float32)
        boff = 0
        for t in range(NT):
            for j in range(NTAB):
                nbj = int(nblk[t][j])
                if nbj == 0:
                    continue
                kk = t * NTAB + j
                cntk = int(kcnt[t][j])
                npad = nbj * 128
                sp = np.zeros(npad, dtype=np.int16)
                lp = np.full(npad, 200.0, dtype=np.float32)
                sp[:cntk] = esrc_l[kptr[kk]:kptr[kk + 1]]
                lp[:cntk] = eldst_s[kptr[kk]:kptr[kk + 1]]
                # idx wrapped in 16 partitions, replicated x8
                w16 = sp.reshape(npad // 16, 16).T  # [16, cols]
                for gidx in range(8):
                    idx2d[gidx * 16:(gidx + 1) * 16,
                          8 * boff: 8 * boff + npad // 16] = w16
                ldst2d[:, boff: boff + nbj] = lp.reshape(nbj, 128).T
                boff += nbj
        deg2d = deg.reshape(NT, 128).T.astype(np.float32)
        per_core.append({"idx2d": idx2d, "ldst2d": ldst2d, "deg2d": deg2d,
                         "slot_nodes": sn})

    meta = {
        "nblk": nblk.tolist(),
        "tsizes": [TBASE[j + 1] - TBASE[j] for j in range(NTAB)],
    }
    return per_core, meta


def kernel(**inputs):
    global LAST_RESULT, LAST_NC
    per_core, meta = _host_prep(inputs)
    nc = _build(meta)
    LAST_NC = nc

    feats16 = np.asarray(inputs["in_feats"]).astype(np.float16)
    shared = {}
    for j in range(NTAB):
        shared[f"feats{j}"] = np.ascontiguousarray(feats16[TBASE[j]:TBASE[j + 1]])
    for name in ["W_conv", "b_conv", "W_ff1", "b_ff1", "W_ih",
                 "W_hh", "b_ih", "b_hh", "W_ff2", "b_ff2"]:
        shared[name] = np.ascontiguousarray(
            np.asarray(inputs[name]).astype(np.float32))

    in_maps = []
    for c in range(NCORES):
        m = dict(shared)
        m["idx2d"] = per_core[c]["idx2d"]
        m["ldst2d"] = per_core[c]["ldst2d"]
        m["deg2d"] = per_core[c]["deg2d"]
        in_maps.append(m)

    res = run_bass_kernel_spmd(nc, in_maps, list(range(NCORES)),
                               trace=bool(int(os.environ.get("KTRACE", "0"))))
    LAST_RESULT = res

    out_full = np.empty((N, D), dtype=np.float32)
    for c in range(NCORES):
        out_full[per_core[c]["slot_nodes"]] = res.results[c]["out"].astype(np.float32)
    return out_full


# revision 14
# speedup vs baseline: 1.7323x; 1.3610x over previous
"""Trainium2 Bass kernel for nn_Block_2302102471059 (ragged_sequence).

Pipeline per NeuronCore (8-way shard by GRU group ownership):
  - Each core owns 16 of the 128 sequence groups -> 8192 "slots"
    (slot s = l*16 + g_local, node id = seq_ids[g, l]).
  - Graph aggregation (segment mean over in-edges) per owned slot:
    in_feats is host-cast to f16 and split into 3 DRAM tables of <32768
    rows (dma_gather indices are int16); per dst-tile (128 slots) edges
    are bucketed by table on the host and fetched with one bulk
    dma_gather per (tile, table) (<=1024 rows per instruction = SWDGE
    ring capacity), then scatter-reduced into PSUM with one-hot matmuls.
  - The 1/deg mean scaling is folded into the layout transpose: instead
    of transposing with an identity, we matmul with diag(invdeg).
  - conv + ff1 in transposed layout (features on partitions), GRU over
    L=512 steps with batch 16, ff2 + transpose back, f16 stores; the
    host scatters rows into the full [65536, 512] f32 output.
"""

import os
import sys

import numpy as np

sys.path.insert(0, "/opt/trn_rl_repo")

from contextlib import ExitStack

import concourse.bacc as bacc
import concourse.bass as bass
import concourse.tile as tile
from concourse import mybir
from concourse.bass_utils import run_bass_kernel_spmd
from concourse.masks import make_identity

N, D, E, G, L = 65536, 512, 1048576, 128, 512
NCORES = 8
GP = G // NCORES          # 16 groups per core
S = GP * L                # 8192 slots per core
NT = S // 128             # 64 dst-tiles of 128 slots
LCH = 32                  # l-steps per stream chunk
NCH = L // LCH            # 16 chunks
F16 = mybir.dt.float16
F32 = mybir.dt.float32
I16 = mybir.dt.int16
I32 = mybir.dt.int32

# 3 gather tables (int16 index limit 32767)
TBASE = [0, 21846, 43691, 65536]
NTAB = 3

LAST_RESULT = None
LAST_NC = None


def _build(meta):
    """meta: dict with
      nblk[t][j]      blocks (128 rows) per (tile, table)
      tsizes[j]       rows per feats table
    """
    nblk = meta["nblk"]
    tsizes = meta["tsizes"]
    nblk_t = [sum(nblk[t]) for t in range(NT)]       # blocks per tile
    maxnblk = max(nblk_t)
    NBLKSUM = sum(nblk_t)                            # total ldst block-cols
    IDXW = 8 * NBLKSUM                               # idx cols (int16)
    # flat offsets
    blk_off = np.zeros((NT, NTAB), dtype=int)
    idx_off = np.zeros((NT, NTAB), dtype=int)
    acc = 0
    for t in range(NT):
        for j in range(NTAB):
            blk_off[t][j] = acc
            idx_off[t][j] = 8 * acc
            acc += nblk[t][j]

    nc = bacc.Bacc("TRN2", target_bir_lowering=False, debug=False)

    # ---- DRAM I/O ----
    feats = [nc.dram_tensor(f"feats{j}", [tsizes[j], D], F16, kind="ExternalInput")
             for j in range(NTAB)]
    d_wconv = nc.dram_tensor("h_wconv", [128, 2048], F16, kind="ExternalInput")
    d_wff1 = nc.dram_tensor("h_wff1", [128, 2048], F16, kind="ExternalInput")
    d_wff2 = nc.dram_tensor("h_wff2", [128, 2048], F16, kind="ExternalInput")
    d_wihT = nc.dram_tensor("h_wihT", [128, 6144], F16, kind="ExternalInput")
    d_whhT = nc.dram_tensor("h_whhT", [128, 6144], F16, kind="ExternalInput")
    d_bconv = nc.dram_tensor("h_bconv", [128, 4], F32, kind="ExternalInput")
    d_bff1 = nc.dram_tensor("h_bff1", [128, 4], F32, kind="ExternalInput")
    d_bff2 = nc.dram_tensor("h_bff2", [128, 4], F32, kind="ExternalInput")
    d_bsum = nc.dram_tensor("h_bsum", [128, 12], F32, kind="ExternalInput")
    d_bhn = nc.dram_tensor("h_bhn", [128, 64], F16, kind="ExternalInput")
    d_invdeg = nc.dram_tensor("h_invdeg", [128, NT], F32, kind="ExternalInput")
    idx2d = nc.dram_tensor("idx2d", [128, IDXW], I16, kind="ExternalInput")
    ldst2d = nc.dram_tensor("ldst2d", [128, NBLKSUM], F32, kind="ExternalInput")
    out = nc.dram_tensor("out", [S, D], F16, kind="ExternalOutput")

    with tile.TileContext(nc) as tc, ExitStack() as ctx:
        wpool = ctx.enter_context(tc.tile_pool(name="w", bufs=1))
        tmp = ctx.enter_context(tc.tile_pool(name="tmp", bufs=2))
        stage = ctx.enter_context(tc.tile_pool(name="stage", bufs=2))
        ohp = ctx.enter_context(tc.tile_pool(name="oh", bufs=2))
        agghp = ctx.enter_context(tc.tile_pool(name="aggh", bufs=2))
        diagp = ctx.enter_context(tc.tile_pool(name="diag", bufs=2))
        xtp = ctx.enter_context(tc.tile_pool(name="xt", bufs=2))
        ctp = ctx.enter_context(tc.tile_pool(name="ct", bufs=2))
        mtp = ctx.enter_context(tc.tile_pool(name="mt", bufs=2))
        gip = ctx.enter_context(tc.tile_pool(name="gi", bufs=2))
        grup = ctx.enter_context(tc.tile_pool(name="gru", bufs=2))
        outp = ctx.enter_context(tc.tile_pool(name="outw", bufs=2))
        ps_a = ctx.enter_context(tc.tile_pool(name="psa", bufs=1, space="PSUM"))
        ps_mm = ctx.enter_context(tc.tile_pool(name="psmm", bufs=2, space="PSUM"))
        ps_gru = ctx.enter_context(tc.tile_pool(name="psgru", bufs=2, space="PSUM"))
        ps_tr = ctx.enter_context(tc.tile_pool(name="pstr", bufs=1, space="PSUM"))

        # ---- constants / meta / weights (host-prepped layouts) ----
        ident = wpool.tile([128, 128], F16, tag="ident")
        make_identity(nc, ident[:])
        iotaf = wpool.tile([128, 128], F32, tag="iotaf")
        iotai = tmp.tile([128, 128], I32, tag="ioi")
        nc.gpsimd.iota(iotai[:], pattern=[[1, 128]], base=0, channel_multiplier=0)
        nc.vector.tensor_copy(iotaf[:], iotai[:])
        zero64 = wpool.tile([128, 64], F16, tag="zero64")
        nc.vector.memset(zero64[:], 0.0)

        idx_sb = wpool.tile([128, IDXW], I16, tag="idxsb")
        nc.sync.dma_start(out=idx_sb[:], in_=idx2d[:, :])
        ldst_f = wpool.tile([128, NBLKSUM], F32, tag="ldstf")
        nc.sync.dma_start(out=ldst_f[:], in_=ldst2d[:, :])
        invdeg = wpool.tile([128, NT], F32, tag="invdeg")
        nc.sync.dma_start(out=invdeg[:], in_=d_invdeg[:, :])

        def loadw(dram, cols, dt, tag):
            t = wpool.tile([128, cols], dt, tag=tag)
            nc.sync.dma_start(out=t[:], in_=dram[:, :])
            return t

        wconv = loadw(d_wconv, 2048, F16, "wconv")
        wff1 = loadw(d_wff1, 2048, F16, "wff1")
        wff2 = loadw(d_wff2, 2048, F16, "wff2")
        wihT = loadw(d_wihT, 6144, F16, "wihT")
        whhT = loadw(d_whhT, 6144, F16, "whhT")
        bconv = loadw(d_bconv, 4, F32, "bconv")
        bff1 = loadw(d_bff1, 4, F32, "bff1")
        bff2 = loadw(d_bff2, 4, F32, "bff2")
        bsum = loadw(d_bsum, 12, F32, "bsum")
        bhn = loadw(d_bhn, 64, F16, "bhn")

        # GRU hidden ring buffer: 64 l-slots x [4 d-chunks x 16 groups]
        ring = wpool.tile([128, 64 * 64], F16, tag="ring")

        def gather_tile(t):
            """Issue the 3 dma_gathers for tile t into a staging tile;
            returns (st, nb) with blocks of the 3 tables back to back."""
            nb = nblk_t[t]
            st = stage.tile([128, maxnblk * D], F16, tag="st")
            b0 = 0
            for j in range(NTAB):
                nbj = nblk[t][j]
                if nbj == 0:
                    continue
                nidx = 128 * nbj
                nc.gpsimd.dma_gather(
                    out_ap=st[:, b0 * D:(b0 + nbj) * D].rearrange(
                        "p (b c) -> p b c", c=D),
                    in_ap=feats[j][:, :],
                    idxs_ap=idx_sb[:, idx_off[t][j]: idx_off[t][j] + 8 * nbj],
                    num_idxs=nidx,
                    num_idxs_reg=nidx,
                    elem_size=D,
                )
                b0 += nbj
            return st, nb

        def gru_step(t_step, gi_t, gi_base):
            """One GRU step. Critical chain: r-mms -> sigmoid(r) -> rhn ->
            tg -> tanh -> mm_ -> hadd. z-sigmoid, omz, zh run off-chain."""
            if t_step == 0:
                h_prev = zero64[:]
            else:
                o = ((t_step - 1) % 64) * 64
                h_prev = ring[:, o:o + 64]
            psr = ps_gru.tile([128, 64], F32, space="PSUM", tag="psr")
            pszn = ps_gru.tile([128, 128], F32, space="PSUM", tag="pszn")
            # gi/bias preloads (independent of h)
            nc.tensor.matmul(psr[:], ident[:], gi_t[:, gi_base:gi_base + 64],
                             start=True, stop=False)
            nc.tensor.matmul(pszn[:, 0:64], ident[:],
                             gi_t[:, gi_base + 64:gi_base + 128],
                             start=True, stop=False)
            nc.tensor.matmul(pszn[:, 64:128], ident[:], bhn[:],
                             start=True, stop=False)
            # r gates (j 0..3) into psr -> sigmoid(r) fires earliest
            for j in range(4):
                for c in range(4):
                    nc.tensor.matmul(
                        psr[:, j * 16:(j + 1) * 16],
                        whhT[:, c * 1536 + j * 128: c * 1536 + (j + 1) * 128],
                        h_prev[:, c * 16:(c + 1) * 16],
                        start=False, stop=(c == 3))
            for j in range(4, 8):
                for c in range(4):
                    nc.tensor.matmul(
                        pszn[:, (j - 4) * 16:(j - 3) * 16],
                        whhT[:, c * 1536 + j * 128: c * 1536 + (j + 1) * 128],
                        h_prev[:, c * 16:(c + 1) * 16],
                        start=False, stop=(c == 3))
            for j in range(8, 12):
                for c in range(4):
                    nc.tensor.matmul(
                        pszn[:, 64 + (j - 8) * 16:64 + (j - 7) * 16],
                        whhT[:, c * 1536 + j * 128: c * 1536 + (j + 1) * 128],
                        h_prev[:, c * 16:(c + 1) * 16],
                        start=False, stop=(c == 3))
            sigr = grup.tile([128, 64], F16, tag="sigr")
            nc.scalar.activation(sigr[:], psr[:],
                                 mybir.ActivationFunctionType.Sigmoid)
            sigz = grup.tile([128, 64], F16, tag="sigz")
            nc.scalar.activation(sigz[:], pszn[:, 0:64],
                                 mybir.ActivationFunctionType.Sigmoid)
            rhn = grup.tile([128, 64], F16, tag="rhn")
            nc.vector.tensor_mul(rhn[:], sigr[:], pszn[:, 64:128])
            tg = grup.tile([128, 64], F16, tag="tg")
            nc.vector.tensor_add(tg[:], rhn[:], gi_t[:, gi_base + 128:gi_base + 192])
            n_t = grup.tile([128, 64], F16, tag="nt")
            nc.scalar.activation(n_t[:], tg[:], mybir.ActivationFunctionType.Tanh)
            omz = grup.tile([128, 64], F16, tag="omz")
            nc.vector.tensor_scalar(out=omz[:], in0=sigz[:],
                                    scalar1=-1.0, scalar2=1.0,
                                    op0=mybir.AluOpType.mult,
                                    op1=mybir.AluOpType.add)
            zh = grup.tile([128, 64], F16, tag="zh")
            nc.vector.tensor_mul(zh[:], sigz[:], h_prev)
            mm_ = grup.tile([128, 64], F16, tag="mm")
            nc.vector.tensor_mul(mm_[:], omz[:], n_t[:])
            hslot = ring[:, (t_step % 64) * 64:(t_step % 64) * 64 + 64]
            nc.vector.tensor_add(hslot, mm_[:], zh[:])

        def ff2_items(k):
            """Items computing ff2 for chunk k's slots from the ring."""
            l0 = (LCH * k) % 64
            rr = ring[:].rearrange("p (l q) -> p l q", q=64)
            state = {}

            def mk_m(m):
                def it():
                    if "ot" not in state:
                        ot_new = outp.tile([128, 4 * 512], F16, tag="ot")
                        state["ot"] = ot_new
                    ot = state["ot"]
                    ps = ps_mm.tile([128, 512], F32, space="PSUM", tag="ps512")
                    for c in range(4):
                        nc.tensor.matmul(
                            ps[:], wff2[:, c * 512 + m * 128: c * 512 + (m + 1) * 128],
                            rr[:, l0:l0 + LCH, c * 16:(c + 1) * 16],
                            start=(c == 0), stop=(c == 3))
                    nc.scalar.activation(ot[:, m * 512:(m + 1) * 512], ps[:],
                                         mybir.ActivationFunctionType.Identity,
                                         bias=bff2[:, m:m + 1])
                return it

            def mk_q(q):
                def it():
                    ot = state["ot"]
                    orow = outp.tile([128, 512], F16, tag="orow")
                    pt = ps_tr.tile([128, 512], F32, space="PSUM", tag="ptr32")
                    for m in range(4):
                        nc.tensor.matmul(
                            pt[:, m * 128:(m + 1) * 128],
                            ot[:, m * 512 + q * 128: m * 512 + (q + 1) * 128],
                            ident[:], start=True, stop=True)
                    nc.vector.tensor_copy(orow[:, 0:256], pt[:, 0:256])
                    nc.vector.tensor_copy(orow[:, 256:512], pt[:, 256:512])
                    nc.sync.dma_start(
                        out=out[k * 512 + q * 128: k * 512 + (q + 1) * 128, :],
                        in_=orow[:])
                return it

            return [mk_m(m) for m in range(4)] + [mk_q(q) for q in range(4)]

        def chunk_dense_items(k):
            """Items for chunk k: gathers, one-hot agg, conv, ff1, gi."""
            state = {}

            def mk_gather(tt):
                t = 4 * k + tt

                def it():
                    st, nb = gather_tile(t)
                    state[tt] = st
                return it

            def mk_oh(tt, g0):
                t = 4 * k + tt

                def it():
                    st = state[tt]
                    nb = nblk_t[t]
                    if g0 >= nb:
                        return
                    gn = min(3, nb - g0)
                    if ("psa", tt) not in state:
                        psa_new = ps_a.tile([128, 512], F32,
                                            space="PSUM", tag="psa")
                        state[("psa", tt)] = psa_new
                    psa = state[("psa", tt)]
                    oh = ohp.tile([128, 3 * 128], F16, tag="oh")
                    src = ldst_f[:, blk_off[t][0] + g0: blk_off[t][0] + g0 + gn]
                    src3 = bass.AP(src.tensor, src.offset,
                                   [src.ap[0], src.ap[1], [0, 128]])
                    io3 = bass.AP(iotaf[:].tensor, iotaf[:].offset,
                                  [iotaf[:].ap[0], [0, gn], iotaf[:].ap[1]])
                    oh3 = oh[:, 0:gn * 128].rearrange("p (q m) -> p q m", m=128)
                    nc.vector.tensor_tensor(out=oh3, in0=src3, in1=io3,
                                            op=mybir.AluOpType.is_equal)
                    for i in range(gn):
                        b = g0 + i
                        nc.tensor.matmul(
                            psa[:], oh[:, i * 128:(i + 1) * 128],
                            st[:, b * D:(b + 1) * D],
                            start=(b == 0), stop=(b == nb - 1))
                return it

            def mk_aggfin(tt):
                t = 4 * k + tt

                def it():
                    psa = state.pop(("psa", tt))
                    if "xt" not in state:
                        xt_new = xtp.tile([128, 4 * 512], F16, tag="xt")
                        state["xt"] = xt_new
                    xt = state["xt"]
                    aggh = agghp.tile([128, 512], F16, tag="aggh")
                    nc.scalar.activation(aggh[:], psa[:],
                                         mybir.ActivationFunctionType.Copy)
                    diag = diagp.tile([128, 128], F16, tag="diag")
                    nc.vector.tensor_scalar(out=diag[:], in0=ident[:],
                                            scalar1=invdeg[:, t:t + 1],
                                            scalar2=None,
                                            op0=mybir.AluOpType.mult)
                    pt = ps_tr.tile([128, 512], F32, space="PSUM", tag="ptr32")
                    for c in range(4):
                        nc.tensor.matmul(pt[:, c * 128:(c + 1) * 128],
                                         aggh[:, c * 128:(c + 1) * 128], diag[:],
                                         start=True, stop=True)
                    xv = xt[:].rearrange("p (c m) -> p c m", m=512)
                    nc.scalar.activation(xv[:, :, tt * 128:(tt + 1) * 128],
                                         pt[:].rearrange("p (c m) -> p c m", m=128),
                                         mybir.ActivationFunctionType.Copy)
                return it

            def mk_conv(m):
                def it():
                    xt = state["xt"]
                    if "ct" not in state:
                        ct_new = ctp.tile([128, 4 * 512], F16, tag="ct")
                        state["ct"] = ct_new
                    ct = state["ct"]
                    ps = ps_mm.tile([128, 512], F32, space="PSUM", tag="ps512")
                    for c in range(4):
                        nc.tensor.matmul(
                            ps[:],
                            wconv[:, c * 512 + m * 128: c * 512 + (m + 1) * 128],
                            xt[:, c * 512:(c + 1) * 512],
                            start=(c == 0), stop=(c == 3))
                    nc.scalar.activation(ct[:, m * 512:(m + 1) * 512], ps[:],
                                         mybir.ActivationFunctionType.Identity,
                                         bias=bconv[:, m:m + 1])
                return it

            def mk_ff1(m):
                def it():
                    ct = state["ct"]
                    if "mt" not in state:
                        mt_new = mtp.tile([128, 4 * 512], F16, tag="mt")
                        state["mt"] = mt_new
                    mt = state["mt"]
                    ps = ps_mm.tile([128, 512], F32, space="PSUM", tag="ps512")
                    for c in range(4):
                        nc.tensor.matmul(
                            ps[:],
                            wff1[:, c * 512 + m * 128: c * 512 + (m + 1) * 128],
                            ct[:, c * 512:(c + 1) * 512],
                            start=(c == 0), stop=(c == 3))
                    nc.scalar.activation(mt[:, m * 512:(m + 1) * 512], ps[:],
                                         mybir.ActivationFunctionType.Relu,
                                         bias=bff1[:, m:m + 1])
                return it

            def mk_gi(j):
                def it():
                    mt = state["mt"]
                    if "git" not in state:
                        git_new = gip.tile([128, LCH * 192], F16, tag="git")
                        state["git"] = git_new
                        gi_tiles[k] = git_new
                    git = state["git"]
                    gir = git[:].rearrange("p (l j b) -> p l j b", j=12, b=16)
                    ps = ps_mm.tile([128, 512], F32, space="PSUM", tag="ps512")
                    for c in range(4):
                        nc.tensor.matmul(
                            ps[:],
                            wihT[:, c * 1536 + j * 128: c * 1536 + (j + 1) * 128],
                            mt[:, c * 512:(c + 1) * 512],
                            start=(c == 0), stop=(c == 3))
                    nc.scalar.activation(
                        gir[:, :, j, :],
                        ps[:].rearrange("p (l b) -> p l b", b=16),
                        mybir.ActivationFunctionType.Identity,
                        bias=bsum[:, j:j + 1])
                return it

            items = []
            items.append(mk_gather(0))
            items.append(mk_gather(1))
            for g0 in range(0, maxnblk, 3):
                items.append(mk_oh(0, g0))
            items.append(mk_aggfin(0))
            items.append(mk_gather(2))
            for g0 in range(0, maxnblk, 3):
                items.append(mk_oh(1, g0))
            items.append(mk_aggfin(1))
            items.append(mk_gather(3))
            for g0 in range(0, maxnblk, 3):
                items.append(mk_oh(2, g0))
            items.append(mk_aggfin(2))
            for g0 in range(0, maxnblk, 3):
                items.append(mk_oh(3, g0))
            items.append(mk_aggfin(3))
            for m in range(4):
                items.append(mk_conv(m))
            for m in range(4):
                items.append(mk_ff1(m))
            for j in range(12):
                items.append(mk_gi(j))
            return items

        # ================= software-pipelined main loop =================
        from collections import deque
        gi_tiles = {}
        pending = deque()
        pending.extend(chunk_dense_items(0))
        while pending:
            pending.popleft()()
        for k in range(NCH):
            if k + 1 < NCH:
                pending.extend(chunk_dense_items(k + 1))
            if k >= 1:
                pending.extend(ff2_items(k - 1))
            per_slot = max(1, -(-len(pending) // LCH))
            git = gi_tiles.pop(k)
            for li in range(LCH):
                gru_step(k * LCH + li, git, li * 192)
                for _ in range(per_slot):
                    if pending:
                        pending.popleft()()
            while pending:
                pending.popleft()()
        for it in ff2_items(NCH - 1):
            it()

    nc.compile()
    return nc


def _host_prep(inputs):
    """Bucket edges by (dst tile, src table) per core; build per-core arrays."""
    seq_ids = np.asarray(inputs["seq_ids"]).astype(np.int64)
    edge_src = np.asarray(inputs["edge_src"]).astype(np.int64)
    edge_dst = np.asarray(inputs["edge_dst"]).astype(np.int64)

    counts = np.bincount(edge_dst, minlength=N)
    order = np.argsort(edge_dst, kind="stable")
    src_sorted = edge_src[order].astype(np.int32)
    rowptr = np.zeros(N + 1, dtype=np.int64)
    np.cumsum(counts, out=rowptr[1:])

    tb = np.asarray(TBASE)

    per_core_raw = []
    cnt_blocks = np.zeros((NCORES, NT, NTAB), dtype=np.int64)
    for c in range(NCORES):
        sn = seq_ids[c * GP:(c + 1) * GP, :].T.reshape(-1)  # [S] slot->node
        deg = counts[sn]
        starts = rowptr[sn]
        tot = int(deg.sum())
        # segmented arange gather of all edges in slot order
        csum = np.cumsum(deg) - deg
        seg = np.arange(tot, dtype=np.int64) - np.repeat(csum, deg)
        esrc = src_sorted[np.repeat(starts, deg) + seg]
        slot_ids = np.repeat(np.arange(S, dtype=np.int64), deg)
        eldst = (slot_ids % 128).astype(np.int32)
        etile = slot_ids // 128
        etab = np.searchsorted(tb[1:NTAB], esrc, side="right")
        key = etile * NTAB + etab
        o2 = np.argsort(key, kind="stable")
        esrc_l = (esrc[o2] - tb[etab[o2]]).astype(np.int16)
        eldst_s = eldst[o2]
        key_s = key[o2]
        kcnt = np.bincount(key_s, minlength=NT * NTAB).reshape(NT, NTAB)
        cnt_blocks[c] = (kcnt + 127) // 128
        per_core_raw.append((sn, deg, esrc_l, eldst_s, kcnt))

    nblk = cnt_blocks.max(axis=0)  # [NT, NTAB]
    nblk_t = nblk.sum(axis=1)
    NBLKSUM = int(nblk_t.sum())
    IDXW = 8 * NBLKSUM

    per_core = []
    for c in range(NCORES):
        sn, deg, esrc_l, eldst_s, kcnt = per_core_raw[c]
        kptr = np.zeros(NT * NTAB + 1, dtype=np.int64)
        np.cumsum(kcnt.reshape(-1), out=kptr[1:])
        idx2d = np.zeros((128, IDXW), dtype=np.int16)
        ldst2d = np.full((128, NBLKSUM), 200.0, dtype=np.float32)
        boff = 0
        for t in range(NT):
            for j in range(NTAB):
                nbj = int(nblk[t][j])
                if nbj == 0:
                    continue
                kk = t * NTAB + j
                cntk = int(kcnt[t][j])
                npad = nbj * 128
                sp = np.zeros(npad, dtype=np.int16)
                lp = np.full(npad, 200.0, dtype=np.float32)
                sp[:cntk] = esrc_l[kptr[kk]:kptr[kk + 1]]
                lp[:cntk] = eldst_s[kptr[kk]:kptr[kk + 1]]
                # idx wrapped in 16 partitions, replicated x8
                w16 = sp.reshape(npad // 16, 16).T  # [16, cols]
                for gidx in range(8):
                    idx2d[gidx * 16:(gidx + 1) * 16,
                          8 * boff: 8 * boff + npad // 16] = w16
                ldst2d[:, boff: boff + nbj] = lp.reshape(nbj, 128).T
                boff += nbj
        invdeg = (1.0 / np.maximum(deg, 1)).reshape(NT, 128).T.astype(np.float32)
        per_core.append({"idx2d": idx2d, "ldst2d": ldst2d, "invdeg": invdeg,
                         "slot_nodes": sn})

    meta = {
        "nblk": nblk.tolist(),
        "tsizes": [TBASE[j + 1] - TBASE[j] for j in range(NTAB)],
    }
    return per_core, meta


def kernel(**inputs):
    global LAST_RESULT, LAST_NC
    per_core, meta = _host_prep(inputs)
    nc = _build(meta)
    LAST_NC = nc

    feats16 = np.asarray(inputs["in_feats"]).astype(np.float16)
    shared = {}
    for j in range(NTAB):
        shared[f"feats{j}"] = np.ascontiguousarray(feats16[TBASE[j]:TBASE[j + 1]])

    def prep_sq(w):  # [512,512] -> [128, c*512+m] f16
        w = np.asarray(w, dtype=np.float32)
        return np.ascontiguousarray(
            w.reshape(4, 128, 512).transpose(1, 0, 2).reshape(128, 2048)
        ).astype(np.float16)

    def prep_gate(w):  # [1536,512] -> [128, c*1536 + j*128 + m] f16
        w = np.asarray(w, dtype=np.float32)
        a = w.reshape(12, 128, 4, 128).transpose(3, 2, 0, 1)  # [p, c, j, m]
        return np.ascontiguousarray(a.reshape(128, 6144)).astype(np.float16)

    def prep_bias(b, n):  # [n*128] -> [128, n] f32
        return np.ascontiguousarray(
            np.asarray(b, dtype=np.float32).reshape(n, 128).T)

    b_ih = np.asarray(inputs["b_ih"], dtype=np.float32)
    b_hh = np.asarray(inputs["b_hh"], dtype=np.float32)
    bsum = prep_bias(b_ih, 12).copy()
    bsum[:, 0:8] += prep_bias(b_hh, 12)[:, 0:8]
    bhn = np.repeat(b_hh[1024:].reshape(4, 128).T[:, :, None], 16,
                    axis=2).transpose(0, 1, 2)  # [p, c, 16]
    bhn = np.ascontiguousarray(bhn.reshape(128, 64)).astype(np.float16)

    shared["h_wconv"] = prep_sq(inputs["W_conv"])
    shared["h_wff1"] = prep_sq(inputs["W_ff1"])
    shared["h_wff2"] = prep_sq(inputs["W_ff2"])
    shared["h_wihT"] = prep_gate(inputs["W_ih"])
    shared["h_whhT"] = prep_gate(inputs["W_hh"])
    shared["h_bconv"] = prep_bias(inputs["b_conv"], 4)
    shared["h_bff1"] = prep_bias(inputs["b_ff1"], 4)
    shared["h_bff2"] = prep_bias(inputs["b_ff2"], 4)
    shared["h_bsum"] = np.ascontiguousarray(bsum)
    shared["h_bhn"] = bhn

    in_maps = []
    for c in range(NCORES):
        m = dict(shared)
        m["idx2d"] = per_core[c]["idx2d"]
        m["ldst2d"] = per_core[c]["ldst2d"]
        m["h_invdeg"] = per_core[c]["invdeg"]
        in_maps.append(m)

    res = run_bass_kernel_spmd(nc, in_maps, list(range(NCORES)),
                               trace=bool(int(os.environ.get("KTRACE", "0"))))
    LAST_RESULT = res

    out_full = np.empty((N, D), dtype=np.float32)
    for c in range(NCORES):
        out_full[per_core[c]["slot_nodes"]] = res.results[c]["out"].astype(np.float32)
    return out_full
